# revision 1
# baseline (speedup 1.0000x reference)
"""Multi-head self-attention with RoPE on 8 Trainium2 NeuronCores.

Sharding: core c = batch*2 + head_group. Each core computes 8 of the 16
heads for one batch element end-to-end (QKV projection, RoPE, attention,
out-projection partial). Host sums the two head-group partials per batch
and applies the (linear) bias corrections.

All matmuls run in bf16 with fp32 PSUM accumulation. Softmax skips the
max-subtraction (scores for this problem are bounded by a few units, so
exp() is safe in fp32) and folds the row-sum into the P@V matmul via a
ones-column on V; normalization happens on the tiny [64, t] output.
"""

import numpy as np
import ml_dtypes

# ---------------------------------------------------------------------------
# Workaround: this walrus build rejects >1 sem-wait on a CTRL-only (Drain)
# instruction. TileContext's tail drain carries one wait per outstanding
# logical proc; split them across a chain of single-wait drains.
# ---------------------------------------------------------------------------
_PATCHED = False


def _split_waits_json(raw: bytes) -> bytes:
    """Split instructions carrying >1 sem-wait into single-wait NoOp
    carriers followed by the original instruction (this walrus build
    allows at most one sync-wait per instruction)."""
    import json

    m = json.loads(raw)

    def fix_block(bb):
        insts = bb.get("instructions")
        if not isinstance(insts, list):
            return
        out = []
        for inst in insts:
            si = inst.get("sync_info") if isinstance(inst, dict) else None
            waits = si.get("on_wait") if si else None
            if waits and len(waits) > 1:
                for k, w in enumerate(waits[:-1]):
                    out.append({
                        "debug": inst.get("debug"),
                        "engine": inst["engine"],
                        "ins": [], "outs": [],
                        "name": f'{inst["name"]}_wc{k}',
                        "opcode": "NoOp",
                        "sync_info": {"on_update": [], "on_wait": [w]},
                        "text_hint": "waitsplit",
                    })
                si["on_wait"] = [waits[-1]]
            out.append(inst)
        bb["instructions"] = out

    def walk(obj):
        if isinstance(obj, dict):
            if "instructions" in obj:
                fix_block(obj)
            for v in obj.values():
                walk(v)
        elif isinstance(obj, list):
            for v in obj:
                walk(v)

    walk(m)
    return json.dumps(m).encode()


def _apply_tile_patch():
    global _PATCHED
    if _PATCHED:
        return
    import concourse.bass as bass

    orig = bass.Bass.to_json_bytes

    def to_json_bytes_split(self, *a, **kw):
        return _split_waits_json(orig(self, *a, **kw))

    bass.Bass.to_json_bytes = to_json_bytes_split
    _PATCHED = True


# ---------------------------------------------------------------------------
# Problem dims (hardcoded for the full problem; parameterized for testing)
# ---------------------------------------------------------------------------
class Cfg:
    def __init__(self, T=2048, CIN=1024, JH=512, CO=1024, D=64):
        self.T, self.CIN, self.JH, self.CO, self.D = T, CIN, JH, CO, D
        self.H = JH // D            # heads per core
        self.NCC = CIN // 128       # contraction chunks
        self.NJ = JH // 128         # q/k row tiles
        self.NT = T // 128          # t partition tiles (= s chunks)
        self.TC = 512               # matmul moving-dim chunk
        self.NTC = T // self.TC
        self.TH = 1024              # exp granule width (2 psum banks)
        self.NTH = T // self.TH
        assert JH % 128 == 0 and CIN % 128 == 0 and T % self.TH == 0
        assert D == 64, "RoPE layout assumes D=64 (pairs at +-32 partitions)"


def rope_tables(cfg, dtype=np.float32):
    """cos/sin tables laid out for the [j-within-tile, t] orientation.

    Partition p of a q/k row-tile holds head-channel d = p % 64; the RoPE
    pair of d is d^32 within the same 64-block. sin is sign-baked:
    negative for the first half of each head, positive for the second.
    """
    half = cfg.D // 2
    theta = (10000.0 ** (-np.arange(half, dtype=np.float32) / half)).astype(np.float32)
    t = np.arange(cfg.T, dtype=np.float32)
    freqs = t[None, :] * theta[:, None]          # (32, T) fp32, matches reference
    cos32, sin32 = np.cos(freqs), np.sin(freqs)
    cos = np.tile(cos32, (4, 1))                 # (128, T)
    sgn = np.where((np.arange(128) % 64) < 32, -1.0, 1.0).astype(np.float32)
    sin = np.tile(sin32, (4, 1)) * sgn[:, None]
    return cos.astype(dtype), sin.astype(dtype)


def perm_matrix():
    """[128,128] permutation: out[p] = in[sigma(p)], sigma(p) = p^32 in 64-blocks."""
    m = np.zeros((128, 128), dtype=np.float32)
    k = np.arange(128)
    sigma = (k // 64) * 64 + (k + 32) % 64
    m[k, sigma] = 1.0
    return m.astype(ml_dtypes.bfloat16)


# ---------------------------------------------------------------------------
# Bass program
# ---------------------------------------------------------------------------
def build_nc(cfg, with_qk_bias=False):
    _apply_tile_patch()
    import concourse.bass as bass
    import concourse.tile as tile
    from concourse import mybir
    import contextlib

    f32 = mybir.dt.float32
    bf16 = mybir.dt.bfloat16
    nc = bass.Bass()

    xT = nc.dram_tensor("xT", (cfg.CIN, cfg.T), bf16, kind="ExternalInput")
    wqT = nc.dram_tensor("wqT", (cfg.CIN, cfg.JH), bf16, kind="ExternalInput")
    wkT = nc.dram_tensor("wkT", (cfg.CIN, cfg.JH), bf16, kind="ExternalInput")
    wvT = nc.dram_tensor("wvT", (cfg.CIN, cfg.JH), bf16, kind="ExternalInput")
    woT = nc.dram_tensor("woT", (cfg.JH, cfg.CO), bf16, kind="ExternalInput")
    cosT = nc.dram_tensor("cosT", (128, cfg.T), f32, kind="ExternalInput")
    sinT = nc.dram_tensor("sinT", (128, cfg.T), f32, kind="ExternalInput")
    permM = nc.dram_tensor("permM", (128, 128), bf16, kind="ExternalInput")
    if with_qk_bias:
        bqD = nc.dram_tensor("bq", (cfg.NJ, 128), f32, kind="ExternalInput")
        bkD = nc.dram_tensor("bk", (cfg.NJ, 128), f32, kind="ExternalInput")
    y = nc.dram_tensor("y", (cfg.T, cfg.CO), f32, kind="ExternalOutput")

    NCC, NJ, NT, TC, NTC, TH, NTH, H = (
        cfg.NCC, cfg.NJ, cfg.NT, cfg.TC, cfg.NTC, cfg.TH, cfg.NTH, cfg.H)

    with tile.TileContext(nc) as tc:
        with contextlib.ExitStack() as ctx:
            consts = ctx.enter_context(tc.tile_pool(name="consts", bufs=1))
            slabs = ctx.enter_context(tc.tile_pool(name="slabs", bufs=1))
            evac = ctx.enter_context(tc.tile_pool(name="evac", bufs=3))
            ropetmp = ctx.enter_context(tc.tile_pool(name="ropetmp", bufs=4))
            ppool = ctx.enter_context(tc.tile_pool(name="ppool", bufs=3))
            ypool = ctx.enter_context(tc.tile_pool(name="ypool", bufs=2))
            rpool = ctx.enter_context(tc.tile_pool(name="rpool", bufs=2))
            rdram = ctx.enter_context(tc.tile_pool(name="rdram", bufs=2, space="DRAM"))

            # ---- constants ----
            cos_sb = consts.tile([128, cfg.T], f32)
            sin_sb = consts.tile([128, cfg.T], f32)
            perm_sb = consts.tile([128, 128], bf16)
            nc.sync.dma_start(out=perm_sb, in_=permM[:, :])
            if with_qk_bias:
                bq_sb = consts.tile([128, NJ], f32)
                bk_sb = consts.tile([128, NJ], f32)
                # dram (NJ, 128) -> sbuf [128, NJ]
                nc.sync.dma_start(out=bq_sb, in_=bqD[:, :].rearrange("j p -> p j"))
                nc.sync.dma_start(out=bk_sb, in_=bkD[:, :].rearrange("j p -> p j"))

            # ---- weight / activation slabs (DMA order = first-use order) ----
            w_sbs = {}
            for name in ("q", "k", "v"):
                w_sbs[name] = slabs.tile([128, NCC, cfg.JH], bf16, tag=f"w{name}",
                                         name=f"w{name}_sb")
            x_sb = slabs.tile([128, NCC, cfg.T], bf16)
            xT_v = xT[:, :].rearrange("(cc p) t -> p cc t", p=128)
            nc.sync.dma_start(out=w_sbs["q"],
                              in_=wqT[:, :].rearrange("(cc p) j -> p cc j", p=128))
            nc.sync.dma_start(out=x_sb[:, :, 0:TC], in_=xT_v[:, :, 0:TC])
            nc.sync.dma_start(out=w_sbs["k"],
                              in_=wkT[:, :].rearrange("(cc p) j -> p cc j", p=128))
            nc.sync.dma_start(out=cos_sb, in_=cosT[:, :])
            nc.sync.dma_start(out=sin_sb, in_=sinT[:, :])
            nc.sync.dma_start(out=w_sbs["v"],
                              in_=wvT[:, :].rearrange("(cc p) j -> p cc j", p=128))
            for tq in range(1, NTC):
                tql = slice(tq * TC, (tq + 1) * TC)
                nc.sync.dma_start(out=x_sb[:, :, tql], in_=xT_v[:, :, tql])
            wo_sb = slabs.tile([128, NJ, cfg.CO], bf16)
            nc.sync.dma_start(out=wo_sb, in_=woT[:, :].rearrange("(jc p) o -> p jc o", p=128))

            qr_sb = slabs.tile([128, NJ, cfg.T], bf16, tag="qr")
            kr_sb = slabs.tile([128, NJ, cfg.T], bf16, tag="kr")
            v_sb = slabs.tile([128, NT, H, cfg.D + 1], bf16, tag="vaug")
            ao_sb = slabs.tile([128, NJ, cfg.T], bf16, tag="ao")
            # ones column for the rowsum trick
            nc.gpsimd.memset(v_sb[:, :, :, cfg.D:cfg.D + 1], 1.0)

            # ================= phase 1: projections + RoPE =================
            with tc.tile_pool(name="pqk", bufs=2, space="PSUM") as pqk, \
                 tc.tile_pool(name="pperm", bufs=2, space="PSUM") as pperm, \
                 tc.tile_pool(name="pv", bufs=2, space="PSUM") as pv:
                for tcc in range(NTC):
                    tsl = slice(tcc * TC, (tcc + 1) * TC)
                    for jt in range(NJ):
                        for name, dst in (("q", qr_sb), ("k", kr_sb)):
                            w = w_sbs[name]
                            ps = pqk.tile([128, TC], f32, tag="pqk")
                            for cc in range(NCC):
                                nc.tensor.matmul(
                                    ps, lhsT=w[:, cc, jt * 128:(jt + 1) * 128],
                                    rhs=x_sb[:, cc, tsl],
                                    start=(cc == 0), stop=(cc == NCC - 1))
                            if with_qk_bias:
                                b = bq_sb if name == "q" else bk_sb
                                nc.vector.tensor_scalar_add(ps, ps, b[:, jt:jt + 1])
                            qb = evac.tile([128, TC], bf16, tag="qb")
                            nc.vector.tensor_copy(qb, ps)
                            pp = pperm.tile([128, TC], f32, tag="pperm")
                            nc.tensor.matmul(pp, lhsT=perm_sb, rhs=qb)
                            t1 = ropetmp.tile([128, TC], f32, tag="t1")
                            nc.vector.tensor_mul(t1, ps, cos_sb[:, tsl])
                            t2 = ropetmp.tile([128, TC], f32, tag="t2")
                            nc.vector.tensor_mul(t2, pp, sin_sb[:, tsl])
                            nc.vector.tensor_add(dst[:, jt, tsl], t1, t2)
                    # v (natural orientation) for this t-chunk's 4 tiles
                    for tt in range(tcc * (TC // 128), (tcc + 1) * (TC // 128)):
                        ps = pv.tile([128, cfg.JH], f32, tag="pv")
                        for cc in range(NCC):
                            nc.tensor.matmul(
                                ps, lhsT=x_sb[:, cc, tt * 128:(tt + 1) * 128],
                                rhs=w_sbs["v"][:, cc, :],
                                start=(cc == 0), stop=(cc == NCC - 1))
                        nc.vector.tensor_copy(
                            v_sb[:, tt, :, 0:cfg.D],
                            ps[:, :].rearrange("p (h d) -> p h d", h=H))

            # ========= phase 2+3: attention (head-pair packed) + out-proj =========
            # psum budget: psc [128,1024] bufs=2 (4 banks, shared with the
            # out-proj y tiles via same tag) + pav [65,512] bufs=4 (4 banks).
            with tc.tile_pool(name="psc", bufs=2, space="PSUM") as psc, \
                 tc.tile_pool(name="pav", bufs=4, space="PSUM") as pav:
                def emit_outproj(tcq, sub=None):
                    tts = range(tcq * (TC // 128), (tcq + 1) * (TC // 128))
                    if sub is not None:
                        tts = [tcq * (TC // 128) + sub]
                    for tt in tts:
                        ps = psc.tile([128, cfg.CO], f32, tag="psc",
                                      name=f"yps_{tt}")
                        for u in range(cfg.CO // TC):
                            for jc in range(NJ):
                                nc.tensor.matmul(
                                    ps[:, u * TC:(u + 1) * TC],
                                    lhsT=ao_sb[:, jc, tt * 128:(tt + 1) * 128],
                                    rhs=wo_sb[:, jc, u * TC:(u + 1) * TC],
                                    start=(jc == 0), stop=(jc == NJ - 1))
                        yb = ypool.tile([128, cfg.CO], f32, tag="yb",
                                        name=f"yb_{tt}")
                        nc.vector.tensor_copy(yb, ps)
                        nc.sync.dma_start(out=y[tt * 128:(tt + 1) * 128, :], in_=yb)

                for tcq in range(NTC):
                    tsl = slice(tcq * TC, (tcq + 1) * TC)
                    if tcq >= 2:
                        emit_outproj(tcq - 2)
                    for pair in range(H // 2):
                        jt = pair
                        avs = [pav.tile([cfg.D + 1, TC], f32, tag="av",
                                        name=f"av_{tcq}_{pair}_{i}")
                               for i in range(2)]
                        for sc in range(NT):
                            pairP = psc.tile([128, 2 * TC], f32, tag="psc")
                            for half in range(2):
                                p0 = 64 * half
                                nc.tensor.matmul(
                                    pairP[:, half * TC:(half + 1) * TC],
                                    lhsT=kr_sb[p0:p0 + 64, jt,
                                               sc * 128:(sc + 1) * 128],
                                    rhs=qr_sb[p0:p0 + 64, jt, tsl],
                                    tile_position=(p0, 0))
                            p_sb = ppool.tile([128, 2 * TC], bf16, tag="p")
                            nc.scalar.activation(
                                p_sb, pairP, mybir.ActivationFunctionType.Exp,
                                scale=float(1.0 / np.sqrt(cfg.D)))
                            for half in range(2):
                                nc.tensor.matmul(
                                    avs[half],
                                    lhsT=v_sb[:, sc, 2 * pair + half, :],
                                    rhs=p_sb[:, half * TC:(half + 1) * TC],
                                    start=(sc == 0), stop=(sc == NT - 1))
                        for half in range(2):
                            av = avs[half]
                            p0 = 64 * half
                            # normalize: ao[d, t] = av[d, t] / av[D, t]
                            r = rpool.tile([1, TC], f32, tag="r")
                            nc.vector.reciprocal(r, av[cfg.D:cfg.D + 1, :])
                            rd = rdram.tile([1, TC], f32, tag="rd")
                            nc.sync.dma_start(out=rd, in_=r)
                            rd_ap = rd[0:1, :]
                            r_bc = bass.AP(
                                tensor=rd_ap.tensor, offset=rd_ap.offset,
                                ap=[[0, cfg.D]] + [list(d) for d in rd_ap.ap[1:]])
                            rb = rpool.tile([cfg.D, TC], f32, tag="rb")
                            nc.sync.dma_start(out=rb, in_=r_bc)
                            nc.vector.tensor_mul(
                                ao_sb[p0:p0 + 64, jt, tsl], av[0:cfg.D, :], rb)
                emit_outproj(NTC - 2)
                emit_outproj(NTC - 1)

    return nc


_NC_CACHE = {}


def _get_nc(cfg, with_qk_bias):
    key = (cfg.T, cfg.CIN, cfg.JH, cfg.CO, cfg.D, with_qk_bias)
    if key not in _NC_CACHE:
        _NC_CACHE[key] = build_nc(cfg, with_qk_bias)
    return _NC_CACHE[key]


def make_in_maps(cfg, x, Wq, bq, Wk, bk, Wv, bv, Wo, bo, n_groups=2):
    """Build the per-core input dicts. Core c = b * n_groups + g."""
    bf = ml_dtypes.bfloat16
    B = x.shape[0]
    cos, sin = rope_tables(cfg)
    pm = perm_matrix()
    with_qk_bias = bool(np.any(bq) or np.any(bk))
    in_maps = []
    for b in range(B):
        for g in range(n_groups):
            rows = slice(g * cfg.JH, (g + 1) * cfg.JH)
            m = {
                "xT": np.ascontiguousarray(x[b].T).astype(bf),
                "wqT": np.ascontiguousarray(Wq[rows, :].T).astype(bf),
                "wkT": np.ascontiguousarray(Wk[rows, :].T).astype(bf),
                "wvT": np.ascontiguousarray(Wv[rows, :].T).astype(bf),
                "woT": np.ascontiguousarray(Wo[:, rows].T).astype(bf),
                "cosT": cos, "sinT": sin, "permM": pm,
            }
            if with_qk_bias:
                m["bq"] = np.ascontiguousarray(
                    bq[rows].reshape(cfg.NJ, 128).astype(np.float32))
                m["bk"] = np.ascontiguousarray(
                    bk[rows].reshape(cfg.NJ, 128).astype(np.float32))
            in_maps.append(m)
    return in_maps, with_qk_bias


def run(x, Wq, bq, Wk, bk, Wv, bv, Wo, bo, trace=False):
    from concourse.bass_utils import run_bass_kernel_spmd

    B, T, C = x.shape
    n_groups = 2
    cfg = Cfg(T=T, CIN=C, JH=C // n_groups, CO=C, D=64)
    in_maps, with_qk_bias = make_in_maps(
        cfg, x, Wq, bq, Wk, bk, Wv, bv, Wo, bo, n_groups)
    nc = _get_nc(cfg, with_qk_bias)
    res = run_bass_kernel_spmd(
        nc, in_maps, core_ids=list(range(len(in_maps))), trace=trace)
    out = np.zeros((B, T, C), dtype=np.float32)
    for c, r in enumerate(res.results):
        out[c // n_groups] += r["y"]
    # linear bias corrections (exact): v-bias passes through softmax row-sum=1;
    # out-proj bias is additive.
    out += (bv.astype(np.float32) @ Wo.T.astype(np.float32) + bo.astype(np.float32))
    return out, res


def kernel(x, Wq, bq, Wk, bk, Wv, bv, Wo, bo):
    out, _ = run(
        np.asarray(x, dtype=np.float32),
        np.asarray(Wq, dtype=np.float32), np.asarray(bq, dtype=np.float32),
        np.asarray(Wk, dtype=np.float32), np.asarray(bk, dtype=np.float32),
        np.asarray(Wv, dtype=np.float32), np.asarray(bv, dtype=np.float32),
        np.asarray(Wo, dtype=np.float32), np.asarray(bo, dtype=np.float32))
    return out

